# revision 1
# baseline (speedup 1.0000x reference)
"""GAT multi-head attention (nn_GATMHAEfficient) on 8 Trainium2 NeuronCores.

Strategy (data-parallel over batch B=32 -> 4 graphs per core):
  Host folds W/Wal/War into one weight matrix Wcat (128 x 152):
    columns [h*17 .. h*17+15] = W[h] (16 cols), column h*17+16 = 0 (later
    memset to 1.0 on-chip -> the "ones" column that makes the aggregation
    matmul also produce the softmax denominator), columns 136..144 = W@Wal
    per head (gives a_i directly from h), columns 144..152 = W@War (a_j).
  Per graph b:  X = h_b @ Wcat  on PE  ->  g, a_i, a_j in one pass.
  Per (b, head): scores are built in (j, i) layout so the aggregation
  matmul can stream them as the moving operand (f32r -> 1 PE cycle/row):
    t   = mask^T * (-1e30) + broadcast(a_i) [+ a_j]
    t   = leaky_relu(t [+ a_j])    alpha=0.2   (ACT Prelu or DVE STT)
    P   = Exp(t)                               (ACT, full tile)
    U^T = sum_c  [g|1]_c^T @ P_c               (PE, PSUM accumulate)
    out = relu(U[:, 0:16] * (1/U[:, 16]))      (PE transpose + DVE)
  The elementwise score work is the bottleneck, so it is load-balanced
  across ACT / DVE / POOL with a per-(b,h) "flavor":
    F1: mask+bcast merge on DVE,  leaky on ACT
    F2: mask+bcast merge on POOL, leaky on ACT
    F3: bcast+aj on DVE(2x), mask merge on POOL, leaky on DVE
"""

import json

import numpy as np

import concourse.bass as bass
import concourse.mybir as mybir
import concourse.tile as tile
from concourse.vector_clock import ScopedClock, VectorClock

F32 = mybir.dt.float32
F32R = mybir.dt.float32r
U8 = mybir.dt.uint8
BF16 = mybir.dt.bfloat16
AF = mybir.ActivationFunctionType
ALU = mybir.AluOpType

B, N, NI, H, D = 32, 1024, 128, 8, 16
NCORES = 8
B_SH = B // NCORES          # graphs per core
C = N // 128                # j-chunks of 128
NEG_SLOPE = 0.2
WCOLS = H * (D + 1) + 2 * H  # 152
MASK_PEN = -1.0e30

# flavor pattern over the 32 (b,h) pairs; tuned from TimelineSim profiles
def _mk_flavors():
    # flavor = (who adds a_j, who does leaky, who applies the mask)
    # F4 : fused into ACT prelu | ACT | POOL
    # F6 : fused into ACT prelu | ACT | DVE
    # F5 : DVE ts               | DVE | POOL
    # F3 : DVE ts               | DVE | DVE
    base = ["F4", "F5", "F6", "F5", "F4", "F3", "F5", "F4",
            "F5", "F3", "F4", "F6", "F5", "F4", "F5", "F3"]
    return [base[i % len(base)] for i in range(32)]

FLAVOR_PATTERN = _mk_flavors()

# ---------------------------------------------------------------------------
# Workarounds for this container's walrus build: it accepts at most ONE
# sync-wait per instruction, but Tile's sem-assignment (and its final drain)
# attach several. Split the excess onto dedicated single-wait EventSemaphore
# carrier instructions in the serialized BIR.


def _legalize_sync_waits(d, max_waits=1):
    for fn in d["functions"]:
        for bb in fn["blocks"]:
            new_insts = []
            for inst in bb["instructions"]:
                si = inst.get("sync_info") or {}
                w = si.get("on_wait") or []
                if len(w) > max_waits:
                    for k, we in enumerate(w[:-max_waits]):
                        new_insts.append(
                            {
                                "debug": inst.get("debug", 0),
                                "engine": inst["engine"],
                                "ins": [],
                                "outs": [],
                                "name": f"{inst['name']}_xw{k}",
                                "opcode": "EventSemaphore",
                                "sync_info": {"on_update": [], "on_wait": [we]},
                            }
                        )
                    si["on_wait"] = w[-max_waits:]
                new_insts.append(inst)
            bb["instructions"] = new_insts


def _wrap_to_json(nc):
    raw = nc.to_json_bytes

    def patched():
        d = json.loads(raw())
        _legalize_sync_waits(d)
        return json.dumps(d).encode()

    nc.to_json_bytes = patched


def _split_drain_and_barrier(self, tick_clock, wait_clock):
    # One drain per logical processor so each carries a single sem wait.
    gc = tick_clock.global_clock
    n = len(gc)
    for proc in range(n):
        t = gc[proc]
        if t > 0:
            dr = self.nc.sync.drain()
            pc = VectorClock([t if i == proc else 0 for i in range(n)])
            wait_clock.add_sem_waits(dr.ins, ScopedClock({None: pc}))
    self.nc.all_engine_barrier()
    popped = self.nc._tile_sem_poison_stack.pop()
    assert popped is self._sem_poison
    self.nc.clear_and_free_semaphores(list(self.sems.allocated().values()))
    self.nc.all_engine_barrier()


tile.TileContext._drain_and_barrier = _split_drain_and_barrier

# ---------------------------------------------------------------------------


def _bcast_rep_ap(t, reps):
    """View a [128, F] tile as [128, reps, F] with a 0-stride middle dim."""
    return bass.AP(tensor=t.tensor, offset=t.offset, ap=[t.ap[0], [0, reps], t.ap[-1]])


def build_nc():
    nc = bass.Bass()
    hT = nc.dram_tensor("hT", [B_SH, NI, N], F32, kind="ExternalInput")
    notmT = nc.dram_tensor("notmT", [B_SH, N, N], BF16, kind="ExternalInput")
    wcat = nc.dram_tensor("wcat", [NI, WCOLS], F32, kind="ExternalInput")
    id17 = nc.dram_tensor("id17", [D + 1, D + 1], F32, kind="ExternalInput")
    out = nc.dram_tensor("out", [B_SH, N, H * D], F32, kind="ExternalOutput")
    ai_scr = nc.dram_tensor("ai_scr", [B_SH, H, N], F32)  # internal scratch

    from contextlib import ExitStack

    with ExitStack() as ctx:
        tc = ctx.enter_context(tile.TileContext(nc))
        const_p = ctx.enter_context(tc.tile_pool(name="const", bufs=1))
        hb_p = ctx.enter_context(tc.tile_pool(name="hb", bufs=2))
        xs_p = ctx.enter_context(tc.tile_pool(name="xs", bufs=2))
        ai_p = ctx.enter_context(tc.tile_pool(name="ai", bufs=2))
        bc_p = ctx.enter_context(tc.tile_pool(name="bc", bufs=4))
        sc_p = ctx.enter_context(tc.tile_pool(name="sc", bufs=7))
        ut_p = ctx.enter_context(tc.tile_pool(name="ut", bufs=2))
        rc_p = ctx.enter_context(tc.tile_pool(name="rc", bufs=2))
        ob_p = ctx.enter_context(tc.tile_pool(name="ob", bufs=2))
        nm_p = ctx.enter_context(tc.tile_pool(name="nm", bufs=2))
        xps_p = ctx.enter_context(tc.tile_pool(name="xps", bufs=2, space="PSUM"))
        aps_p = xps_p
        vps_p = xps_p
        ups_p = ctx.enter_context(tc.tile_pool(name="ups", bufs=2, space="PSUM"))
        if True:
            wcat_s = const_p.tile([NI, WCOLS], F32)
            nc.sync.dma_start(out=wcat_s[:], in_=wcat[:])
            id17_s = const_p.tile([D + 1, D + 1], F32)
            nc.sync.dma_start(out=id17_s[:], in_=id17[:])
            onec = const_p.tile([128, 1], F32)
            nc.vector.memset(onec[:], 1.0)

            def _prep(b):
                hbT = hb_p.tile([NI, N], F32)
                nc.sync.dma_start(out=hbT[:], in_=hT[b])
                notm = nm_p.tile([128, C, N], BF16)
                nc.sync.dma_start(
                    out=notm[:], in_=notmT[b].rearrange("(c p) i -> p c i", p=128)
                )

                # X = h_b @ Wcat, one 128-row chunk at a time.
                # g columns land in an f32r tile (the aggregation matmul
                # requires f32r-rounded producers); a_j columns stay fp32.
                GEXT = H * (D + 1)  # 136
                gext_r = xs_p.tile([128, C, GEXT], F32R, tag="gext")
                Xs_aj = xs_p.tile([128, C, H], F32, tag="xsaj")
                for c in range(C):
                    X_ps = xps_p.tile([128, WCOLS], F32, tag="xv")
                    nc.tensor.matmul(
                        X_ps[:],
                        lhsT=hbT[:, c * 128 : (c + 1) * 128],
                        rhs=wcat_s[:],
                        start=True,
                        stop=True,
                    )
                    nc.vector.tensor_copy(gext_r[:, c, :], X_ps[:, 0:GEXT])
                    nc.scalar.copy(out=Xs_aj[:, c, :], in_=X_ps[:, GEXT + H :])
                # ones column per head block (the denominator column of
                # gext); memset can't write f32r, so broadcast-copy from an
                # fp32 constant instead
                ones_view = bass.AP(
                    tensor=gext_r.tensor,
                    offset=gext_r.offset + D,  # first ones slot at col 16
                    ap=[gext_r.ap[0], [GEXT, C], [D + 1, H]],
                )
                ones_src = bass.AP(
                    tensor=onec.tensor,
                    offset=onec.offset,
                    ap=[onec.ap[0], [0, C], [0, H]],
                )
                nc.vector.tensor_copy(ones_view, ones_src)

                # a_i rows for every head: (W@Wal)^T @ h_b -> (8, N),
                # round-tripped through DRAM so each row can be broadcast
                # to all partitions by a replicating DMA.
                XT_ps = aps_p.tile([H, N], F32, tag="xv")
                for half in range(2):
                    sl = slice(half * 512, (half + 1) * 512)
                    nc.tensor.matmul(
                        XT_ps[:, sl],
                        lhsT=wcat_s[:, H * (D + 1) : H * (D + 1) + H],
                        rhs=hbT[:, sl],
                        start=True,
                        stop=True,
                    )
                ais8 = ai_p.tile([H, N], F32)
                nc.vector.tensor_copy(ais8[:], XT_ps[:])
                nc.sync.dma_start(out=ai_scr[b], in_=ais8[:])

                return notm, gext_r, Xs_aj

            preps = {0: _prep(0)}

            for b in range(B_SH):
                notm, gext_r, Xs_aj = preps.pop(b)
                out_b = ob_p.tile([128, C, H * D], F32)

                def postproc(h, UT_ps, flavor):
                        UT_s = ut_p.tile([D + 1, N], F32)
                        if h % 2 == 0:
                            nc.vector.tensor_copy(UT_s[:], UT_ps[:])
                        else:
                            nc.scalar.copy(out=UT_s[:], in_=UT_ps[:])

                        # transpose back to (i, 17) and normalize
                        V_ps = vps_p.tile([128, C, D + 1], F32, tag="xv")
                        for c in range(C):
                            nc.tensor.transpose(
                                V_ps[:, c, :],
                                UT_s[:, c * 128 : (c + 1) * 128],
                                id17_s[:],
                            )
                        rc_s = rc_p.tile([128, C], F32)
                        nc.vector.reciprocal(rc_s[:], V_ps[:, :, D])
                        # batched: u = V * (1/S) broadcast along d, then relu
                        rc_b = bass.AP(
                            tensor=rc_s.tensor,
                            offset=rc_s.offset,
                            ap=[rc_s.ap[0], rc_s.ap[-1], [0, D]],
                        )
                        u_s = rc_p.tile([128, C, D], F32, tag="u_s")
                        nc.vector.tensor_tensor(
                            out=u_s[:], in0=V_ps[:, :, 0:D], in1=rc_b, op=ALU.mult
                        )
                        nc.vector.tensor_scalar(
                            out=out_b[:, :, h * D : (h + 1) * D],
                            in0=u_s[:],
                            scalar1=0.0,
                            scalar2=None,
                            op0=ALU.max,
                        )


                pending = []

                for h in range(H):

                    # broadcast a_i over all 128 partitions (replicating DMA)
                    bc = bc_p.tile([128, N], F32)
                    bcast_src = bass.AP(
                        tensor=ai_scr,
                        offset=(b * H + h) * N,
                        ap=[[0, 128], [1, N]],
                    )
                    nc.sync.dma_start(out=bc[:], in_=bcast_src)

                    def aj_col(c):
                        return bass.AP(
                            tensor=Xs_aj.tensor,
                            offset=Xs_aj.offset + c * H + h,
                            ap=[Xs_aj.ap[0], [1, 1]],
                        )

                    UT_ps = ups_p.tile([D + 1, N], F32)
                    gh = gext_r[:, :, h * (D + 1) : (h + 1) * (D + 1)]
                    # stage-major emission in half-pair groups: each engine
                    # streams its stage back-to-back so chunk stages pipeline
                    GRP = 2
                    for g0 in range(0, C, GRP):
                        cs = list(range(g0, g0 + GRP))
                        flavor = FLAVOR_PATTERN[
                            ((b * H + h) * (C // GRP) + g0 // GRP) % len(FLAVOR_PATTERN)
                        ]
                        # one 2-chunk tile: per-chunk APs for the bias stages,
                        # full-tile (FD 2048) for leaky/exp/mask to halve the
                        # per-op init overhead on ACT/DVE/POOL
                        t2c = sc_p.tile([128, GRP, N], F32, tag="sc", name=f"t_{b}_{h}_{g0}")
                        if flavor in ("F4", "F6"):
                            # t = leaky_relu(bc + a_j) straight from bc on ACT
                            for i, c in enumerate(cs):
                                nc.scalar.activation(
                                    out=t2c[:, i, :],
                                    in_=bc[:],
                                    func=AF.Prelu,
                                    bias=aj_col(c),
                                    scale=1.0,
                                    alpha=NEG_SLOPE,
                                )
                        else:
                            # t = bc + a_j (DVE 2x), then leaky in place (DVE)
                            for i, c in enumerate(cs):
                                nc.vector.tensor_scalar(
                                    out=t2c[:, i, :],
                                    in0=bc[:],
                                    scalar1=aj_col(c),
                                    scalar2=None,
                                    op0=ALU.add,
                                )
                            nc.vector.scalar_tensor_tensor(
                                out=t2c[:],
                                in0=t2c[:],
                                scalar=NEG_SLOPE,
                                in1=t2c[:],
                                op0=ALU.mult,
                                op1=ALU.max,
                            )
                        # P = exp(...) in place  (unmasked), full 2-chunk op
                        nc.scalar.activation(out=t2c[:], in_=t2c[:], func=AF.Exp)
                        # P_m = P * (1 - mask), rounding to f32r, full 2-chunk
                        meng = nc.gpsimd if flavor in ("F4", "F5") else nc.vector
                        pm2 = sc_p.tile([128, GRP, N], F32R, tag="pm", name=f"pm_{b}_{h}_{g0}")
                        meng.tensor_tensor(
                            out=pm2[:],
                            in0=t2c[:],
                            in1=notm[:, g0 : g0 + GRP, :],
                            op=ALU.mult,
                        )
                        for i, c in enumerate(cs):
                            for half in range(2):
                                sl = slice(half * 512, (half + 1) * 512)
                                nc.tensor.matmul(
                                    UT_ps[:, sl],
                                    lhsT=gh[:, c, :],
                                    rhs=pm2[:, i, sl],
                                    start=(c == 0),
                                    stop=(c == C - 1),
                                )
                    # deferred postproc of the previous head overlaps
                    # this head's score stages
                    if pending:
                        postproc(*pending.pop())
                    pending.append((h, UT_ps, None))
                    # emit next graph's prep early so it overlaps this
                    # graph's remaining heads instead of stalling at the
                    # boundary
                    if h == 1 and b + 1 < B_SH:
                        preps[b + 1] = _prep(b + 1)

                if pending:
                    postproc(*pending.pop())
                nc.sync.dma_start(
                    out=out[b].rearrange("(c p) d -> p c d", p=128), in_=out_b[:]
                )

    _wrap_to_json(nc)
    return nc


_NC_CACHE = None


def kernel(h, W, Wal, War, mask):
    global _NC_CACHE
    from concourse.bass_utils import run_bass_kernel_spmd

    h = np.asarray(h, dtype=np.float32)
    W = np.asarray(W, dtype=np.float32)
    Wal = np.asarray(Wal, dtype=np.float32)
    War = np.asarray(War, dtype=np.float32)
    import ml_dtypes

    notm_b16 = (~np.asarray(mask, dtype=bool)).astype(ml_dtypes.bfloat16)

    # Fold weights: wcat = [per-head (W_h | 0)] + [W@Wal] + [W@War]
    wcat = np.zeros((NI, WCOLS), dtype=np.float32)
    for hh in range(H):
        wcat[:, hh * (D + 1) : hh * (D + 1) + D] = W[hh]
        wcat[:, H * (D + 1) + hh] = W[hh] @ Wal[hh, :, 0]
        wcat[:, H * (D + 1) + H + hh] = W[hh] @ War[hh, :, 0]

    hT = np.ascontiguousarray(h.transpose(0, 2, 1))            # (B, I, N)
    notmT = np.ascontiguousarray(notm_b16.transpose(0, 2, 1))  # (B, j, i)
    id17 = np.eye(D + 1, dtype=np.float32)

    if _NC_CACHE is None:
        _NC_CACHE = build_nc()
    nc = _NC_CACHE

    in_maps = []
    for core in range(NCORES):
        sl = slice(core * B_SH, (core + 1) * B_SH)
        in_maps.append(
            {
                "hT": np.ascontiguousarray(hT[sl]),
                "notmT": np.ascontiguousarray(notmT[sl]),
                "wcat": wcat,
                "id17": id17,
            }
        )

    res = run_bass_kernel_spmd(nc, in_maps, list(range(NCORES)))
    out = np.concatenate([res.results[i]["out"] for i in range(NCORES)], axis=0)
    return out.astype(np.float32)



# revision 2
# speedup vs baseline: 1.3090x; 1.3090x over previous
"""GAT multi-head attention (nn_GATMHAEfficient) on 8 Trainium2 NeuronCores.

Data-parallel over batch B=32 -> 4 graphs per core. Host folds W/Wal/War
into wcat (128 x 152): per-head [W_h | 0] blocks (the 0 column becomes the
on-chip "ones" column so the aggregation matmul also produces the softmax
denominator), then W@Wal (a_i) and W@War (a_j) columns.

Score pipeline: exp(leaky_relu(a_i + a_j)) == max(E_i*E_j, F_i*F_j) with
E = exp(a), F = exp(0.2*a) (exact identity: for s>0 exp(s) wins, for s<0
exp(0.2s) wins). The exps move to the small per-node vectors, so the N^2
work is cheap bf16 ALU ops instead of ACT table ops:
  u  = (Ei_bc * Ej_ptr)            DVE tensor_scalar (bf16 4x mode)
  w  = (Fi_bc * Fj_ptr) max u      Pool STT (fused) or DVE TS+TT
  P  = w * notm                    DVE tensor_tensor (bf16 2x mode)
A tunable fraction of tiles instead uses the direct ACT pipeline
(Prelu(ai_bc + aj_bias) in f32, Exp -> bf16) to load-balance ACT vs
DVE vs Pool. P (bf16) streams into the PE aggregation matmul at
1 cycle/row; per-head [g|1] stationary gives numerator + denominator.
"""

import json

import numpy as np

import concourse.bass as bass
import concourse.mybir as mybir
import concourse.tile as tile
from concourse.vector_clock import ScopedClock, VectorClock

F32 = mybir.dt.float32
BF16 = mybir.dt.bfloat16
AF = mybir.ActivationFunctionType
ALU = mybir.AluOpType

B, N, NI, H, D = 32, 1024, 128, 8, 16
NCORES = 8
B_SH = B // NCORES          # graphs per core
C = N // 128                # j-chunks of 128
NEG_SLOPE = 0.2
GEXT = H * (D + 1)          # 136: per-head [g(16) | ones]
WCOLS = GEXT + 2 * H        # 152: + a_i cols + a_j cols
GRP = 2                     # chunks per score tile
NG = C // GRP               # score tile groups per (b, h)

# flavor pattern over groups: OLD = ACT prelu+exp, NP = DVE-TS + Pool-STT,
# ND = all-DVE. Tuned so ACT/DVE/Pool busy times balance.
PATTERN = ["OLD", "NP", "NP", "OLD", "NP", "NP", "OLD", "ND",
           "OLD", "NP", "NP", "OLD", "NP", "NP", "OLD", "NP"]

# ---------------------------------------------------------------------------
# Workarounds for this container's walrus build: it accepts at most ONE
# sync-wait per instruction, but Tile's sem-assignment (and its final drain)
# attach several. Split the excess onto dedicated single-wait EventSemaphore
# carrier instructions in the serialized BIR.


def _legalize_sync_waits(d, max_waits=1):
    for fn in d["functions"]:
        for bb in fn["blocks"]:
            new_insts = []
            for inst in bb["instructions"]:
                si = inst.get("sync_info") or {}
                w = si.get("on_wait") or []
                if len(w) > max_waits:
                    for k, we in enumerate(w[:-max_waits]):
                        new_insts.append(
                            {
                                "debug": inst.get("debug", 0),
                                "engine": inst["engine"],
                                "ins": [],
                                "outs": [],
                                "name": f"{inst['name']}_xw{k}",
                                "opcode": "EventSemaphore",
                                "sync_info": {"on_update": [], "on_wait": [we]},
                            }
                        )
                    si["on_wait"] = w[-max_waits:]
                new_insts.append(inst)
            bb["instructions"] = new_insts


def _wrap_to_json(nc):
    raw = nc.to_json_bytes

    def patched():
        d = json.loads(raw())
        _legalize_sync_waits(d)
        return json.dumps(d).encode()

    nc.to_json_bytes = patched


def _split_drain_and_barrier(self, tick_clock, wait_clock):
    # One drain per logical processor so each carries a single sem wait.
    gc = tick_clock.global_clock
    n = len(gc)
    for proc in range(n):
        t = gc[proc]
        if t > 0:
            dr = self.nc.sync.drain()
            pc = VectorClock([t if i == proc else 0 for i in range(n)])
            wait_clock.add_sem_waits(dr.ins, ScopedClock({None: pc}))
    self.nc.all_engine_barrier()
    popped = self.nc._tile_sem_poison_stack.pop()
    assert popped is self._sem_poison
    self.nc.clear_and_free_semaphores(list(self.sems.allocated().values()))
    self.nc.all_engine_barrier()


tile.TileContext._drain_and_barrier = _split_drain_and_barrier

# ---------------------------------------------------------------------------


def build_nc():
    nc = bass.Bass()
    hT = nc.dram_tensor("hT", [B_SH, NI, N], F32, kind="ExternalInput")
    notmT = nc.dram_tensor("notmT", [B_SH, N, N], BF16, kind="ExternalInput")
    wcat = nc.dram_tensor("wcat", [NI, WCOLS], F32, kind="ExternalInput")
    id17 = nc.dram_tensor("id17", [D + 1, D + 1], F32, kind="ExternalInput")
    out = nc.dram_tensor("out", [B_SH, N, H * D], F32, kind="ExternalOutput")
    # per (b,h): rows [Ei | Fi | ai] staged for the broadcast DMA
    ef_scr = nc.dram_tensor("ef_scr", [B_SH, H, 3, N], BF16)

    from contextlib import ExitStack

    with ExitStack() as ctx:
        tc = ctx.enter_context(tile.TileContext(nc))
        const_p = ctx.enter_context(tc.tile_pool(name="const", bufs=1))
        hb_p = ctx.enter_context(tc.tile_pool(name="hb", bufs=2))
        nm_p = ctx.enter_context(tc.tile_pool(name="nm", bufs=2))
        gx_p = ctx.enter_context(tc.tile_pool(name="gx", bufs=2))
        aj_p = ctx.enter_context(tc.tile_pool(name="aj", bufs=2))
        ei_p = ctx.enter_context(tc.tile_pool(name="ei", bufs=2))
        bc_p = ctx.enter_context(tc.tile_pool(name="bc", bufs=3))
        sc_p = ctx.enter_context(tc.tile_pool(name="sc", bufs=3))
        ut_p = ctx.enter_context(tc.tile_pool(name="ut", bufs=2))
        rc_p = ctx.enter_context(tc.tile_pool(name="rc", bufs=2))
        ob_p = ctx.enter_context(tc.tile_pool(name="ob", bufs=2))
        xps_p = ctx.enter_context(tc.tile_pool(name="xps", bufs=2, space="PSUM"))
        ups_p = ctx.enter_context(tc.tile_pool(name="ups", bufs=2, space="PSUM"))
        if True:
            wcat_s = const_p.tile([NI, WCOLS], F32)
            nc.sync.dma_start(out=wcat_s[:], in_=wcat[:])
            id17_s = const_p.tile([D + 1, D + 1], F32)
            nc.sync.dma_start(out=id17_s[:], in_=id17[:])
            onec = const_p.tile([128, 1], BF16)
            nc.vector.memset(onec[:], 1.0)

            def _prep(b):
                hbT = hb_p.tile([NI, N], F32)
                nc.sync.dma_start(out=hbT[:], in_=hT[b])
                notm = nm_p.tile([128, C, N], BF16)
                nc.sync.dma_start(
                    out=notm[:], in_=notmT[b].rearrange("(c p) i -> p c i", p=128)
                )

                # X = h_b @ wcat chunk by chunk; g -> bf16, a_j cols -> f32
                gext = gx_p.tile([128, C, GEXT], BF16, tag="gx")
                aj_s = aj_p.tile([128, C, H], F32, tag="aj")
                ej_s = aj_p.tile([128, C, H], F32, tag="ej")
                fj_s = aj_p.tile([128, C, H], F32, tag="fj")
                for c in range(C):
                    X_ps = xps_p.tile([128, WCOLS], F32, tag="xv")
                    nc.tensor.matmul(
                        X_ps[:],
                        lhsT=hbT[:, c * 128 : (c + 1) * 128],
                        rhs=wcat_s[:],
                        start=True,
                        stop=True,
                    )
                    if c % 2 == 0:
                        nc.vector.tensor_copy(gext[:, c, :], X_ps[:, 0:GEXT])
                    else:
                        nc.scalar.copy(out=gext[:, c, :], in_=X_ps[:, 0:GEXT])
                    nc.scalar.copy(out=aj_s[:, c, :], in_=X_ps[:, GEXT + H : WCOLS])
                # per-head ones column (bf16 1.0 exact)
                ones_view = bass.AP(
                    tensor=gext.tensor,
                    offset=gext.offset + D,
                    ap=[gext.ap[0], [GEXT, C], [D + 1, H]],
                )
                ones_src = bass.AP(
                    tensor=onec.tensor,
                    offset=onec.offset,
                    ap=[onec.ap[0], [0, C], [0, H]],
                )
                nc.vector.tensor_copy(ones_view, ones_src)
                # E_j = exp(a_j), F_j = exp(0.2 a_j) per-partition scalars
                nc.scalar.activation(out=ej_s[:], in_=aj_s[:], func=AF.Exp, scale=1.0)
                nc.scalar.activation(out=fj_s[:], in_=aj_s[:], func=AF.Exp, scale=0.2)

                # a_i rows for all heads, then E_i/F_i/a_i staged to DRAM for
                # the partition-broadcast DMA
                XT_ps = xps_p.tile([H, N], F32, tag="xv")
                for half in range(2):
                    sl = slice(half * 512, (half + 1) * 512)
                    nc.tensor.matmul(
                        XT_ps[:, sl],
                        lhsT=wcat_s[:, GEXT : GEXT + H],
                        rhs=hbT[:, sl],
                        start=True,
                        stop=True,
                    )
                Ei8 = ei_p.tile([H, N], BF16, tag="ei8")
                Fi8 = ei_p.tile([H, N], BF16, tag="fi8")
                ai8 = ei_p.tile([H, N], BF16, tag="ai8")
                nc.scalar.activation(out=Ei8[:], in_=XT_ps[:], func=AF.Exp, scale=1.0)
                nc.scalar.activation(out=Fi8[:], in_=XT_ps[:], func=AF.Exp, scale=0.2)
                nc.vector.tensor_copy(ai8[:], XT_ps[:])
                for k, t8 in enumerate((Ei8, Fi8, ai8)):
                    dst = bass.AP(
                        tensor=ef_scr,
                        offset=(b * H * 3 + k) * N,
                        ap=[[3 * N, H], [1, N]],
                    )
                    nc.sync.dma_start(out=dst, in_=t8[:])

                return notm, gext, aj_s, ej_s, fj_s

            preps = {0: _prep(0)}

            for b in range(B_SH):
                notm, gext, aj_s, ej_s, fj_s = preps.pop(b)
                out_b = ob_p.tile([128, C, H * D], F32)

                def postproc(h, UT_ps, _):
                    UT_s = ut_p.tile([D + 1, N], F32)
                    if h % 2 == 0:
                        nc.vector.tensor_copy(UT_s[:], UT_ps[:])
                    else:
                        nc.scalar.copy(out=UT_s[:], in_=UT_ps[:])

                    # transpose back to (i, 17) and normalize
                    V_ps = xps_p.tile([128, C, D + 1], F32, tag="xv")
                    for c in range(C):
                        nc.tensor.transpose(
                            V_ps[:, c, :],
                            UT_s[:, c * 128 : (c + 1) * 128],
                            id17_s[:],
                        )
                    rc_s = rc_p.tile([128, C], F32)
                    nc.vector.reciprocal(rc_s[:], V_ps[:, :, D])
                    rc_b = bass.AP(
                        tensor=rc_s.tensor,
                        offset=rc_s.offset,
                        ap=[rc_s.ap[0], rc_s.ap[-1], [0, D]],
                    )
                    u_s = rc_p.tile([128, C, D], F32, tag="u_s")
                    nc.vector.tensor_tensor(
                        out=u_s[:], in0=V_ps[:, :, 0:D], in1=rc_b, op=ALU.mult
                    )
                    nc.vector.tensor_scalar(
                        out=out_b[:, :, h * D : (h + 1) * D],
                        in0=u_s[:],
                        scalar1=0.0,
                        scalar2=None,
                        op0=ALU.max,
                    )

                pending = []

                for h in range(H):
                    # one DMA broadcasts [Ei | Fi | ai] rows to all partitions
                    bc3 = bc_p.tile([128, 3, N], BF16)
                    nc.sync.dma_start(
                        out=bc3[:],
                        in_=bass.AP(
                            tensor=ef_scr,
                            offset=(b * H + h) * 3 * N,
                            ap=[[0, 128], [1, 3 * N]],
                        ),
                    )
                    Ei_bc = bc3[:, 0, :]
                    Fi_bc = bc3[:, 1, :]
                    ai_bc = bc3[:, 2, :]

                    def col(t, c):
                        return bass.AP(
                            tensor=t.tensor,
                            offset=t.offset + c * H + h,
                            ap=[t.ap[0], [1, 1]],
                        )

                    UT_ps = ups_p.tile([D + 1, N], F32)
                    gh = gext[:, :, h * (D + 1) : (h + 1) * (D + 1)]
                    for gi, g0 in enumerate(range(0, C, GRP)):
                        cs = list(range(g0, g0 + GRP))
                        flavor = PATTERN[((b * H + h) * NG + gi) % len(PATTERN)]
                        pm = sc_p.tile([128, GRP, N], BF16, tag="pm", name=f"pm_{b}_{h}_{g0}")
                        if flavor == "OLD":
                            t2 = sc_p.tile([128, GRP, N], F32, tag="t2", name=f"t2_{b}_{h}_{g0}")
                            for i, c in enumerate(cs):
                                nc.scalar.activation(
                                    out=t2[:, i, :],
                                    in_=ai_bc,
                                    func=AF.Prelu,
                                    bias=col(aj_s, c),
                                    scale=1.0,
                                    alpha=NEG_SLOPE,
                                )
                            ex = sc_p.tile([128, GRP, N], BF16, tag="u", name=f"ex_{b}_{h}_{g0}")
                            nc.scalar.activation(out=ex[:], in_=t2[:], func=AF.Exp)
                            nc.vector.tensor_tensor(
                                out=pm[:], in0=ex[:], in1=notm[:, g0 : g0 + GRP, :],
                                op=ALU.mult,
                            )
                        else:
                            u2 = sc_p.tile([128, GRP, N], BF16, tag="u", name=f"u_{b}_{h}_{g0}")
                            for i, c in enumerate(cs):
                                nc.vector.tensor_scalar(
                                    out=u2[:, i, :],
                                    in0=Ei_bc,
                                    scalar1=col(ej_s, c),
                                    scalar2=None,
                                    op0=ALU.mult,
                                )
                            if flavor == "NP":
                                for i, c in enumerate(cs):
                                    nc.gpsimd.scalar_tensor_tensor(
                                        out=u2[:, i, :],
                                        in0=Fi_bc,
                                        scalar=col(fj_s, c),
                                        in1=u2[:, i, :],
                                        op0=ALU.mult,
                                        op1=ALU.max,
                                    )
                            else:  # ND
                                v2 = sc_p.tile([128, GRP, N], BF16, tag="v", name=f"v_{b}_{h}_{g0}")
                                for i, c in enumerate(cs):
                                    nc.vector.tensor_scalar(
                                        out=v2[:, i, :],
                                        in0=Fi_bc,
                                        scalar1=col(fj_s, c),
                                        scalar2=None,
                                        op0=ALU.mult,
                                    )
                                nc.vector.tensor_tensor(
                                    out=u2[:], in0=u2[:], in1=v2[:], op=ALU.max
                                )
                            nc.vector.tensor_tensor(
                                out=pm[:], in0=u2[:], in1=notm[:, g0 : g0 + GRP, :],
                                op=ALU.mult,
                            )
                        for i, c in enumerate(cs):
                            for half in range(2):
                                sl = slice(half * 512, (half + 1) * 512)
                                nc.tensor.matmul(
                                    UT_ps[:, sl],
                                    lhsT=gh[:, c, :],
                                    rhs=pm[:, i, sl],
                                    start=(c == 0),
                                    stop=(c == C - 1),
                                )
                    # deferred postproc of the previous head overlaps this
                    # head's score stages
                    if pending:
                        postproc(*pending.pop())
                    pending.append((h, UT_ps, None))
                    if h == 1 and b + 1 < B_SH:
                        preps[b + 1] = _prep(b + 1)

                if pending:
                    postproc(*pending.pop())
                nc.sync.dma_start(
                    out=out[b].rearrange("(c p) d -> p c d", p=128), in_=out_b[:]
                )

    _wrap_to_json(nc)
    return nc


_NC_CACHE = None


def kernel(h, W, Wal, War, mask):
    global _NC_CACHE
    from concourse.bass_utils import run_bass_kernel_spmd

    h = np.asarray(h, dtype=np.float32)
    W = np.asarray(W, dtype=np.float32)
    Wal = np.asarray(Wal, dtype=np.float32)
    War = np.asarray(War, dtype=np.float32)
    import ml_dtypes

    notm_b16 = (~np.asarray(mask, dtype=bool)).astype(ml_dtypes.bfloat16)

    # Fold weights: wcat = [per-head (W_h | 0)] + [W@Wal] + [W@War]
    wcat = np.zeros((NI, WCOLS), dtype=np.float32)
    for hh in range(H):
        wcat[:, hh * (D + 1) : hh * (D + 1) + D] = W[hh]
        wcat[:, GEXT + hh] = W[hh] @ Wal[hh, :, 0]
        wcat[:, GEXT + H + hh] = W[hh] @ War[hh, :, 0]

    hT = np.ascontiguousarray(h.transpose(0, 2, 1))            # (B, I, N)
    notmT = np.ascontiguousarray(notm_b16.transpose(0, 2, 1))  # (B, j, i)
    id17 = np.eye(D + 1, dtype=np.float32)

    if _NC_CACHE is None:
        _NC_CACHE = build_nc()
    nc = _NC_CACHE

    in_maps = []
    for core in range(NCORES):
        sl = slice(core * B_SH, (core + 1) * B_SH)
        in_maps.append(
            {
                "hT": np.ascontiguousarray(hT[sl]),
                "notmT": np.ascontiguousarray(notmT[sl]),
                "wcat": wcat,
                "id17": id17,
            }
        )

    res = run_bass_kernel_spmd(nc, in_maps, list(range(NCORES)))
    out = np.concatenate([res.results[i]["out"] for i in range(NCORES)], axis=0)
    return out.astype(np.float32)
